# revision 5
# baseline (speedup 1.0000x reference)
"""Trainium2 Bass kernel for nn_CriticEstimator.

Math (per batch row b, agent n):
    x    = [state, action]                      # [192]
    e    = x @ G_n + gb_n                       # [256]
    v    = relu(e @ W_V^T)                      # [256]
    (the reference's attention einsum 'bhnm,bhnd->bhnd' multiplies v by
     softmax row-sums == 1, so the attention block is an exact pass-through)
    h    = relu([e, v] @ W1_n + b1_n)           # [256]
    q    = h @ w2_n + b2_n                      # [1]

Since e has no nonlinearity, fold it into the weights (host-side, fp64):
    A_n = G_n @ W_V^T            v = relu(x @ A_n + c_v),  c_v = gb_n @ W_V^T
    P_n = G_n @ U1_n             h = relu(x @ P_n + v @ U2_n + c_h)
                                 c_h = gb_n @ U1_n + b1_n
    (U1_n = W1_n[:256], U2_n = W1_n[256:])
b2 is added on the host after gathering.

Device layout: feature-major activations (x^T shipped from host), so every
matmul contracts over SBUF partitions with zero on-chip transposes.  Agents
are sharded 2-per-core across 8 cores; each core streams the full batch in
512-column tiles.  Matmuls run in fp16 (fp32 PSUM accumulate).
"""

import sys

if "/opt/trn_rl_repo" not in sys.path:
    sys.path.insert(0, "/opt/trn_rl_repo")

import numpy as np

import concourse.bass as bass
import concourse.mybir as mybir
from concourse import bacc
from concourse.tile import TileContext
from concourse.bass_utils import run_bass_kernel_spmd

B, N, S, A, E = 8192, 16, 128, 64, 256
IN = S + A                     # 192
NCORES = 8
G = N // NCORES                # agents per core
BT = 512                       # batch columns per tile (one PSUM bank)
NBT = B // BT
K0 = 128                       # x-feature chunk 0: features 0..127 (state)
F32 = mybir.dt.float32
F32R = mybir.dt.float32r
F16 = mybir.dt.float16
RELU = mybir.ActivationFunctionType.Relu
ADD = mybir.AluOpType.add
MAX = mybir.AluOpType.max
MULT = mybir.AluOpType.mult

_BUILT = {}
MODE = "pair"  # "pair": row-tiled K=64 action matmuls; "pad": zero-padded K=128
XPACK = False  # pack xk0+xk1 tiles into one contiguous DMA per batch tile
ABLATE = ""
MMDT = F16    # matmul operand dtype
NPDT = np.float16   # host-side dtype matching MMDT


def _build(repeats=1):
    key = (repeats, MODE, XPACK, ABLATE)
    if key in _BUILT:
        return _BUILT[key]

    nc = bacc.Bacc("TRN2", target_bir_lowering=False, debug=False,
                   num_devices=NCORES)

    if XPACK:
        xk = nc.dram_tensor("xk", [G, NBT, 128, 2 * BT], MMDT,
                            kind="ExternalInput").ap()
    else:
        xk0 = nc.dram_tensor("xk0", [G, K0, B], MMDT, kind="ExternalInput").ap()
        # action features duplicated across rows 0:64 and 64:128 on the host
        xk1 = nc.dram_tensor("xk1", [G, 128, B], MMDT, kind="ExternalInput").ap()
    wa = nc.dram_tensor("wa", [G, K0, E], MMDT, kind="ExternalInput").ap()
    wp = nc.dram_tensor("wp", [G, K0, E], MMDT, kind="ExternalInput").ap()
    if MODE == "pair":
        # action-chunk weights packed: rows 0:64 = A action part (v stage),
        # rows 64:128 = P action part (h stage); K=64 matmuls on disjoint
        # PE row-tiles (0,0)/(64,0).
        wac = nc.dram_tensor("wac", [G, 128, E], MMDT, kind="ExternalInput").ap()
    elif MODE == "pad2":
        # two separate zero-padded K=128 weight tensors
        wacv = nc.dram_tensor("wacv", [G, 128, E], MMDT, kind="ExternalInput").ap()
        wach = nc.dram_tensor("wach", [G, 128, E], MMDT, kind="ExternalInput").ap()
    else:  # pad / padv / padh share the 2E layout
        # zero-padded K=128 action weights: cols 0:E v-stage (rows 64:128
        # zero), cols E:2E h-stage (rows 0:64 zero)
        wac = nc.dram_tensor("wac", [G, 128, 2 * E], MMDT,
                             kind="ExternalInput").ap()
    wu = nc.dram_tensor("wu", [G, E, E], MMDT, kind="ExternalInput").ap()
    # w2 chunks replicated across 128 output columns
    w2 = nc.dram_tensor("w2", [G, 128, 2 * 128], MMDT, kind="ExternalInput").ap()
    cv = nc.dram_tensor("cv", [G, 128, 2], F32, kind="ExternalInput").ap()
    ch = nc.dram_tensor("ch", [G, 128, 2], F32, kind="ExternalInput").ap()
    y = nc.dram_tensor("y", [G, B], F32, kind="ExternalOutput").ap()

    with TileContext(nc) as tc:
        with (
            tc.tile_pool(name="wpool", bufs=2) as wpool,
            tc.tile_pool(name="xpool", bufs=8) as xpool,
            tc.tile_pool(name="vpool", bufs=4) as vpool,
            tc.tile_pool(name="hpool", bufs=4) as hpool,
            tc.tile_pool(name="qpool", bufs=2) as qpool,
            tc.tile_pool(name="pv", bufs=2, space="PSUM") as pvpool,
            tc.tile_pool(name="ph", bufs=4, space="PSUM") as phpool,
            tc.tile_pool(name="pq", bufs=2, space="PSUM") as pqpool,
            _repeat_loop(tc, repeats),
        ):
            for g in range(G):
                wa_t = wpool.tile([K0, E], MMDT)
                nc.sync.dma_start(out=wa_t[:], in_=wa[g, :, :])
                wp_t = wpool.tile([K0, E], MMDT)
                nc.sync.dma_start(out=wp_t[:], in_=wp[g, :, :])
                if MODE == "pad2":
                    wacv_t = wpool.tile([128, E], MMDT)
                    nc.sync.dma_start(out=wacv_t[:], in_=wacv[g, :, :])
                    wach_t = wpool.tile([128, E], MMDT)
                    nc.sync.dma_start(out=wach_t[:], in_=wach[g, :, :])
                else:
                    wac_t = wpool.tile([128, E * (1 if MODE == "pair" else 2)], MMDT)
                    nc.sync.dma_start(out=wac_t[:], in_=wac[g, :, :])
                wu0_t = wpool.tile([128, E], MMDT)
                nc.sync.dma_start(out=wu0_t[:], in_=wu[g, :128, :])
                wu1_t = wpool.tile([128, E], MMDT)
                nc.sync.dma_start(out=wu1_t[:], in_=wu[g, 128:, :])
                w2_t = wpool.tile([128, 2 * 128], MMDT)
                nc.sync.dma_start(out=w2_t[:], in_=w2[g, :, :])
                cv_t = wpool.tile([128, 2], F32)
                nc.sync.dma_start(out=cv_t[:], in_=cv[g, :, :])
                ch_t = wpool.tile([128, 2], F32)
                nc.sync.dma_start(out=ch_t[:], in_=ch[g, :, :])

                q_sb = qpool.tile([1, B], F32)
                if ABLATE in ("mm_only",):
                    nc.gpsimd.memset(q_sb[:], 0.0)

                ms0, ms1 = slice(0, 128), slice(128, 256)

                prev_hts = None
                for bt in range(NBT):
                    c0, c1 = bt * BT, (bt + 1) * BT
                    if XPACK:
                        xt = xpool.tile([128, 2 * BT], MMDT)
                        nc.sync.dma_start(out=xt[:], in_=xk[g, bt, :, :])
                        xt0 = xt[:, 0:BT]
                        xt1 = xt[:, BT:2 * BT]
                    else:
                        xt0_t = xpool.tile([K0, BT], MMDT)
                        nc.sync.dma_start(out=xt0_t[:], in_=xk0[g, :, c0:c1])
                        xt1_t = xpool.tile([128, BT], MMDT)
                        nc.sync.dma_start(out=xt1_t[:], in_=xk1[g, :, c0:c1])
                        xt0 = xt0_t[:]
                        xt1 = xt1_t[:]

                    # Accumulation chains interleaved across PSUM banks
                    # (same-bank back-to-back matmuls measure slower).
                    pv0 = pvpool.tile([128, BT], F32, name="pv0", bufs=1)
                    pv1 = pvpool.tile([128, BT], F32, name="pv1", bufs=1)
                    ph0 = phpool.tile([128, BT], F32, name="ph0", bufs=2)
                    ph1 = phpool.tile([128, BT], F32, name="ph1", bufs=2)
                    nc.tensor.matmul(pv0[:], wa_t[:, ms0], xt0,
                                     start=True, stop=False)
                    nc.tensor.matmul(pv1[:], wa_t[:, ms1], xt0,
                                     start=True, stop=False)
                    if MODE == "pair":
                        nc.tensor.matmul(pv0[:], wac_t[0:64, ms0], xt1[0:64, :],
                                         start=False, stop=True,
                                         tile_position=(0, 0))
                        nc.tensor.matmul(ph0[:], wac_t[64:128, ms0],
                                         xt1[64:128, :],
                                         start=True, stop=False,
                                         tile_position=(64, 0))
                        nc.tensor.matmul(pv1[:], wac_t[0:64, ms1], xt1[0:64, :],
                                         start=False, stop=True,
                                         tile_position=(0, 0))
                        nc.tensor.matmul(ph1[:], wac_t[64:128, ms1],
                                         xt1[64:128, :],
                                         start=True, stop=False,
                                         tile_position=(64, 0))
                    elif MODE == "pad2":
                        nc.tensor.matmul(pv0[:], wacv_t[:, ms0], xt1,
                                         start=False, stop=True)
                        nc.tensor.matmul(ph0[:], wach_t[:, ms0], xt1,
                                         start=True, stop=False)
                        nc.tensor.matmul(pv1[:], wacv_t[:, ms1], xt1,
                                         start=False, stop=True)
                        nc.tensor.matmul(ph1[:], wach_t[:, ms1], xt1,
                                         start=True, stop=False)
                    elif MODE == "padv":
                        # v-stage padded K=128, h-stage paired K=64
                        nc.tensor.matmul(pv0[:], wac_t[:, ms0], xt1,
                                         start=False, stop=True)
                        nc.tensor.matmul(ph0[:], wac_t[64:128, E + 0:E + 128],
                                         xt1[64:128, :],
                                         start=True, stop=False,
                                         tile_position=(64, 0))
                        nc.tensor.matmul(pv1[:], wac_t[:, ms1], xt1,
                                         start=False, stop=True)
                        nc.tensor.matmul(ph1[:], wac_t[64:128, E + 128:2 * E],
                                         xt1[64:128, :],
                                         start=True, stop=False,
                                         tile_position=(64, 0))
                    elif MODE == "padh":
                        # v-stage paired K=64, h-stage padded K=128
                        nc.tensor.matmul(pv0[:], wac_t[0:64, ms0], xt1[0:64, :],
                                         start=False, stop=True,
                                         tile_position=(0, 0))
                        nc.tensor.matmul(ph0[:], wac_t[:, E:E + 128], xt1,
                                         start=True, stop=False)
                        nc.tensor.matmul(pv1[:], wac_t[0:64, ms1], xt1[0:64, :],
                                         start=False, stop=True,
                                         tile_position=(0, 0))
                        nc.tensor.matmul(ph1[:], wac_t[:, E + 128:2 * E], xt1,
                                         start=True, stop=False)
                    else:
                        nc.tensor.matmul(pv0[:], wac_t[:, ms0], xt1,
                                         start=False, stop=True)
                        nc.tensor.matmul(ph0[:], wac_t[:, E:E + 128], xt1,
                                         start=True, stop=False)
                        nc.tensor.matmul(pv1[:], wac_t[:, ms1], xt1,
                                         start=False, stop=True)
                        nc.tensor.matmul(ph1[:], wac_t[:, E + 128:2 * E], xt1,
                                         start=True, stop=False)
                    vts = []
                    if ABLATE != "mm_only":
                        vt0 = vpool.tile([128, BT], MMDT, name="vt0")
                        nc.scalar.activation(vt0[:], pv0[:], RELU,
                                             bias=cv_t[:, 0:1])
                        vt1 = vpool.tile([128, BT], MMDT, name="vt1")
                        nc.scalar.activation(vt1[:], pv1[:], RELU,
                                             bias=cv_t[:, 1:2])
                        vts = [vt0, vt1]

                    rhs0 = vts[0][:] if ABLATE != "mm_only" else xt0
                    rhs1 = vts[1][:] if ABLATE != "mm_only" else xt0
                    # previous tile's q chain rides between the h x-part and
                    # v-part, keeping bank alternation and filling PE time
                    # while this tile's v evictions drain
                    if prev_hts is not None:
                        c0p, c1p = (bt - 1) * BT, bt * BT
                        pq = pqpool.tile([128, BT], F32, name="pq", bufs=2)
                        nc.tensor.matmul(pq[:], w2_t[:, 0:128],
                                         prev_hts[0][:],
                                         start=True, stop=False)
                    nc.tensor.matmul(ph0[:], wp_t[:, ms0], xt0,
                                     start=False, stop=False)
                    nc.tensor.matmul(ph1[:], wp_t[:, ms1], xt0,
                                     start=False, stop=False)
                    if prev_hts is not None:
                        nc.tensor.matmul(pq[:], w2_t[:, 128:256],
                                         prev_hts[1][:],
                                         start=False, stop=True)
                        if ABLATE != "mm_only":
                            nc.vector.tensor_copy(q_sb[0:1, c0p:c1p],
                                                  pq[0:1, :])
                    nc.tensor.matmul(ph0[:], wu0_t[:, ms0], rhs0,
                                     start=False, stop=False)
                    nc.tensor.matmul(ph1[:], wu0_t[:, ms1], rhs0,
                                     start=False, stop=False)
                    nc.tensor.matmul(ph0[:], wu1_t[:, ms0], rhs1,
                                     start=False, stop=True)
                    nc.tensor.matmul(ph1[:], wu1_t[:, ms1], rhs1,
                                     start=False, stop=True)
                    hts = []
                    if ABLATE != "mm_only":
                        ht0 = hpool.tile([128, BT], MMDT, name="ht0")
                        nc.vector.tensor_scalar(ht0[:], ph0[:],
                                                ch_t[:, 0:1], 0.0,
                                                op0=ADD, op1=MAX)
                        ht1 = hpool.tile([128, BT], MMDT, name="ht1")
                        nc.scalar.activation(ht1[:], ph1[:], RELU,
                                             bias=ch_t[:, 1:2])
                        hts = [ht0, ht1]
                    prev_hts = hts if ABLATE != "mm_only" else [xt0_t, xt0_t] if not XPACK else [xt, xt]

                if prev_hts is not None and ABLATE != "mm_only":
                    c0p, c1p = (NBT - 1) * BT, NBT * BT
                    pq = pqpool.tile([128, BT], F32, name="pq", bufs=2)
                    nc.tensor.matmul(pq[:], w2_t[:, 0:128], prev_hts[0][:],
                                     start=True, stop=False)
                    nc.tensor.matmul(pq[:], w2_t[:, 128:256], prev_hts[1][:],
                                     start=False, stop=True)
                    nc.vector.tensor_copy(q_sb[0:1, c0p:c1p], pq[0:1, :])
                nc.sync.dma_start(out=y[g, :], in_=q_sb[0:1, :])

    nc.finalize()
    _BUILT[key] = nc
    return nc


def _repeat_loop(tc, repeats):
    # benchmarking aid: run the whole body `repeats` times so wall-clock
    # differences isolate on-device execution time
    from contextlib import nullcontext
    return tc.For_i(0, repeats, 1) if repeats > 1 else nullcontext()


def _prep_in_maps(state, action, g_weight, g_bias, W_Q, W_K, W_V,
                  gx_weight_1, gx_bias_1, gx_weight_2, gx_bias_2):
    f8 = np.float64
    WVt = W_V.T.astype(f8)
    in_maps = []
    for c in range(NCORES):
        if XPACK:
            xk = np.zeros((G, NBT, 128, 2 * BT), NPDT)
        else:
            xk0 = np.empty((G, K0, B), NPDT)
            xk1 = np.empty((G, 128, B), NPDT)
        wa = np.empty((G, K0, E), NPDT)
        wp = np.empty((G, K0, E), NPDT)
        if MODE == "pad2":
            wacv_a = np.zeros((G, 128, E), NPDT)
            wach_a = np.zeros((G, 128, E), NPDT)
        wac = np.zeros((G, 128, E * (1 if MODE == "pair" else 2)), NPDT)
        wu = np.empty((G, E, E), NPDT)
        w2 = np.empty((G, 128, 2 * 128), NPDT)
        cv = np.empty((G, 128, 2), np.float32)
        ch = np.empty((G, 128, 2), np.float32)
        for g in range(G):
            n = c * G + g
            st_T = state[:, n, :].T
            act_T = action[:, n, :].T
            if XPACK:
                for bt in range(NBT):
                    sl = slice(bt * BT, (bt + 1) * BT)
                    xk[g, bt, :, 0:BT] = st_T[:, sl]
                    xk[g, bt, :A, BT:2 * BT] = act_T[:, sl]
                    xk[g, bt, A:, BT:2 * BT] = act_T[:, sl]
            else:
                xk0[g] = st_T
                xk1[g, :A] = act_T
                xk1[g, A:] = act_T
            Gn = g_weight[n].astype(f8)
            U1 = gx_weight_1[n][:E].astype(f8)
            An = (Gn @ WVt).astype(np.float32)
            Pn = (Gn @ U1).astype(np.float32)
            wa[g] = An[:K0]
            wp[g] = Pn[:K0]
            if MODE == "pad2":
                wacv_a[g, :A, :] = An[K0:]
                wach_a[g, A:, :] = Pn[K0:]
            elif MODE == "pair":
                wac[g, :A, 0:E] = An[K0:]
                wac[g, A:, 0:E] = Pn[K0:]
            else:
                wac[g, :A, 0:E] = An[K0:]
                wac[g, A:, E:2 * E] = Pn[K0:]
                if MODE == "padv":
                    # h-stage reads K=64 rows 64:128 of cols E:2E (same bytes)
                    pass
            wu[g] = gx_weight_1[n][E:]
            w2[g, :, 0:128] = gx_weight_2[n][:128, 0:1]
            w2[g, :, 128:256] = gx_weight_2[n][128:, 0:1]
            cvn = (g_bias[n].astype(f8) @ WVt).astype(np.float32)
            chn = (g_bias[n].astype(f8) @ U1
                   + gx_bias_1[n].astype(f8)).astype(np.float32)
            cv[g, :, 0] = cvn[:128]
            cv[g, :, 1] = cvn[128:]
            ch[g, :, 0] = chn[:128]
            ch[g, :, 1] = chn[128:]
        m = {
            "wa": np.ascontiguousarray(wa),
            "wp": np.ascontiguousarray(wp),
            "wu": np.ascontiguousarray(wu),
            "w2": np.ascontiguousarray(w2),
            "cv": np.ascontiguousarray(cv),
            "ch": np.ascontiguousarray(ch),
        }
        if MODE == "pad2":
            m["wacv"] = np.ascontiguousarray(wacv_a)
            m["wach"] = np.ascontiguousarray(wach_a)
        else:
            m["wac"] = np.ascontiguousarray(wac)
        if XPACK:
            m["xk"] = np.ascontiguousarray(xk)
        else:
            m["xk0"] = np.ascontiguousarray(xk0)
            m["xk1"] = np.ascontiguousarray(xk1)
        in_maps.append(m)
    return in_maps


def _run(in_maps, repeats=1, **kwargs):
    nc = _build(repeats)
    return run_bass_kernel_spmd(nc, in_maps, list(range(NCORES)), **kwargs)


def _gather(results, gx_bias_2):
    out = np.empty((B, N, 1), np.float32)
    for c in range(NCORES):
        yc = results[c]["y"]
        for g in range(G):
            n = c * G + g
            out[:, n, 0] = yc[g] + gx_bias_2[n, 0]
    return out


def kernel(**inputs):
    inputs = {k: np.asarray(v) for k, v in inputs.items()}
    in_maps = _prep_in_maps(**inputs)
    res = _run(in_maps)
    return _gather(res.results, inputs["gx_bias_2"])
